# revision 16
# baseline (speedup 1.0000x reference)
"""KroneckerLinear Trainium2 kernel.

Math: out = x @ kron(f1, f2).T + bias, with x [64, 8192], f1 [128,128],
f2 [64,64], bias [8192].  Kronecker identity:
    out[b].reshape(128, 64) = f1 @ X_b @ f2.T,   X_b = x[b].reshape(128, 64)
so the 8192x8192 weight (256 MB) is never materialized; the kernel is
memory-bound on x in / out (~4 MB total).

Sharding: batch-parallel over the 8 NeuronCores, 8 batch rows per core.

Per-core device program (SPMD, identical on all cores):
  loads: matmul operands in bf16 (halves DMA bytes; PE runs 1 cycle/row),
     bias fp32 (riding in the bf16 tensor via a bitcast view; the final
     add runs in fp32 against the fp32 PSUM).  Two parallel DMAs on the
     two HWDGE rings (sync + scalar), each fully contiguous; per-DMA
     completion latency (~1.5us) dominates, so exactly two big DMAs beat
     any finer split.
     A = blk | f1t | xt0 | xt1, B = xt2 | xt3 | biasr.
     xt[h*64+l, p*128+j] = x[lb, j*64+l] for local batch lb = p + 4h;
     blk = blkdiag(f2.T, f2.T) so one K=128 matmul computes TWO batches.
  stage 1 (apply f2): matmul p: lhsT = xt_p, rhs = blk ->
     psum_v[p][j, h*64+k] = (X_{p+4h} @ f2.T)[j, k], one PSUM tile per p
     so each PSUM->SBUF cast only waits on its own matmul.  Casts
     (fp32 PSUM -> bf16 v) alternate scalar (ACTIVATE) / vector (DVE);
     gpsimd can't read PSUM on TRN2.
  stage 2 (apply f1): two matmuls lhsT = f1t, rhs = v half [128, 256].
  bias: fused with the PSUM->SBUF move on DVE, fp32.
  stores: two contiguous fp32 DMAs (sync + scalar rings in parallel) of
     the device-natural layout y[i, p*128+h*64+k]; host unpermutes.
"""

import numpy as np

N_CORES = 8
B = 64
LB = B // N_CORES  # 8 local batches per core

_CACHE = {}


def _build_nc():
    import concourse.bass as bass
    import concourse.mybir as mybir
    import concourse.tile as tile
    from concourse import bacc

    fp32 = mybir.dt.float32
    bf16 = mybir.dt.bfloat16

    nc = bacc.Bacc("TRN2", target_bir_lowering=False, debug=False)
    # contiguous per-ring inputs:
    # A: blk 0:128 | f1t 128:256 | xt0 256:384 | xt1 384:512   (bf16)
    # B: xt2 0:128 | xt3 128:256 | biasr-as-bf16 256:384       (bf16)
    inA_d = nc.dram_tensor("inpA", [128, 512], bf16, kind="ExternalInput")
    inB_d = nc.dram_tensor("inpB", [128, 384], bf16, kind="ExternalInput")
    y0_d = nc.dram_tensor("y0", [128, 256], fp32, kind="ExternalOutput")
    y1_d = nc.dram_tensor("y1", [128, 256], fp32, kind="ExternalOutput")

    with tile.TileContext(nc) as tc:
        with (
            tc.tile_pool(name="sb", bufs=1) as sb,
            tc.tile_pool(name="psv", bufs=1, space="PSUM") as psv,
            tc.tile_pool(name="psy", bufs=1, space="PSUM") as psy,
        ):
            sbA = sb.tile([128, 512], bf16)
            sbB = sb.tile([128, 384], bf16)
            v = sb.tile([128, 512], bf16)
            out0 = sb.tile([128, 256], fp32)
            out1 = sb.tile([128, 256], fp32)
            warm = sb.tile([128, 128], bf16)
            warm_v = sb.tile([128, 128], bf16)

            blk = sbA[:, 0:128]
            f1t = sbA[:, 128:256]
            biasr = sbB[:, 256:384].bitcast(fp32)  # [128, 64] fp32

            def xt_slice(p):
                return (
                    sbA[:, 256 + p * 128 : 384 + p * 128]
                    if p < 2
                    else sbB[:, (p - 2) * 128 : (p - 1) * 128]
                )

            # two parallel input DMAs, one per HWDGE ring
            nc.sync.dma_start(out=sbA[:, :], in_=inA_d[:, :])
            nc.scalar.dma_start(out=sbB[:, :], in_=inB_d[:, :])

            # warm-up: ramp PE/DVE clocks during the ~2.5us DMA-wait window
            # so the first real ops run at full rate instead of cold pstate
            nc.vector.memset(warm[:, :], 0.0)
            pw = psv.tile([128, 128], fp32, tag="psum_warm")
            for _ in range(4):
                nc.tensor.matmul(pw[:], warm[:, :], warm[:, :], start=True, stop=True)
            for _ in range(3):
                nc.vector.tensor_copy(warm_v[:, :], warm[:, :])

            b_ap = biasr
            bias_bcast = bass.AP(
                tensor=b_ap.tensor,
                offset=b_ap.offset,
                ap=[b_ap.ap[0], [0, 4], b_ap.ap[1]],
            )

            # stage 1: 4 matmuls, paired into two PSUM tiles so each
            # PSUM->SBUF cast covers a [128, 256] pair in one DVE op
            psum_v = []
            for pair in range(2):
                pv = psv.tile([128, 256], fp32, tag=f"psum_v{pair}")
                psum_v.append(pv)
                for half in range(2):
                    p = pair * 2 + half
                    nc.tensor.matmul(
                        pv[:, half * 128 : (half + 1) * 128],
                        xt_slice(p),
                        blk,
                        start=True,
                        stop=True,
                    )
            # PSUM -> SBUF casts (fp32 -> bf16), all on DVE: using scalar
            # ACTIVATE would pull in an act-table DMA that contends with the
            # scalar-ring input DMA for HBM bandwidth
            for pair in range(2):
                dst = v[:, pair * 256 : (pair + 1) * 256]
                nc.vector.tensor_copy(dst, psum_v[pair][:])

            # stage 2: two N=256 matmuls
            psum_y = []
            for hlf in range(2):
                py = psy.tile([128, 256], fp32, tag=f"psum_y{hlf}")
                psum_y.append(py)
                nc.tensor.matmul(
                    py[:],
                    f1t,
                    v[:, hlf * 256 : (hlf + 1) * 256],
                    start=True,
                    stop=True,
                )

            # bias add fused with PSUM->SBUF move (DVE, fp32)
            for hlf, out_sb in enumerate([out0, out1]):
                o_ap = out_sb[:, :]
                out_g = bass.AP(
                    tensor=o_ap.tensor,
                    offset=o_ap.offset,
                    ap=[o_ap.ap[0], [64, 4], [1, 64]],
                )
                y_ap = psum_y[hlf][:, :]
                y_g = bass.AP(
                    tensor=y_ap.tensor,
                    offset=y_ap.offset,
                    ap=[y_ap.ap[0], [64, 4], [1, 64]],
                )
                nc.vector.tensor_add(out_g, y_g, bias_bcast)

            # two parallel contiguous output DMAs
            nc.sync.dma_start(out=y0_d[:, :], in_=out0[:, :])
            nc.scalar.dma_start(out=y1_d[:, :], in_=out1[:, :])

    nc.compile()
    return nc


def _prep_core_inputs(x, factor1, factor2, bias):
    """Host-side layout prep. Returns list of per-core in_maps."""
    import ml_dtypes

    bf16 = ml_dtypes.bfloat16
    x = np.ascontiguousarray(np.asarray(x, dtype=np.float32))
    f1 = np.asarray(factor1, dtype=np.float32)
    f2 = np.asarray(factor2, dtype=np.float32)
    bias = np.asarray(bias, dtype=np.float32)

    # x -> per-core xt [128, 512]: xt[h*64+l, p*128+j] = x[c*8 + p + 4h, j*64+l]
    xc = x.reshape(N_CORES, LB, 128, 64)  # [c, lb, j, l]
    arr = xc.transpose(0, 3, 1, 2).reshape(N_CORES, 64, 2, 4, 128)
    xt_all = arr.transpose(0, 2, 1, 3, 4).reshape(N_CORES, 128, 512).astype(bf16)

    inA = np.zeros((N_CORES, 128, 512), dtype=bf16)
    f2t = f2.T.astype(bf16)
    inA[:, :64, 0:64] = f2t
    inA[:, 64:, 64:128] = f2t
    inA[:, :, 128:256] = f1.T.astype(bf16)
    inA[:, :, 256:512] = xt_all[:, :, 0:256]
    inB = np.empty((N_CORES, 128, 384), dtype=bf16)
    inB[:, :, 0:256] = xt_all[:, :, 256:512]
    bias_bf = np.ascontiguousarray(bias.reshape(128, 64)).view(bf16)  # [128,128]
    inB[:, :, 256:384] = bias_bf[None]

    return [
        {"inpA": np.ascontiguousarray(inA[c]), "inpB": np.ascontiguousarray(inB[c])}
        for c in range(N_CORES)
    ]


def kernel(x, factor1, factor2, bias):
    from concourse.bass_utils import run_bass_kernel_spmd

    if "nc" not in _CACHE:
        _CACHE["nc"] = _build_nc()
    nc = _CACHE["nc"]

    in_maps = _prep_core_inputs(x, factor1, factor2, bias)
    res = run_bass_kernel_spmd(nc, in_maps, core_ids=list(range(N_CORES)))
    kernel.last_results = res

    # device layout: y[i, p*128 + h*64 + k] = out[c*8 + p + 4h, i*64 + k]
    # row order after reshape is r = 2p + h; batch lb = p + 4h -> inv perm
    inv = np.array([0, 2, 4, 6, 1, 3, 5, 7])
    outs = []
    for c in range(N_CORES):
        yc = np.concatenate(
            [res.results[c]["y0"], res.results[c]["y1"]], axis=1
        )  # [128, 512]
        yc = yc.reshape(128, 4, 2, 64).transpose(1, 2, 0, 3).reshape(8, 8192)
        outs.append(yc[inv])
    return np.concatenate(outs, axis=0)


# revision 37
# speedup vs baseline: 1.1122x; 1.1122x over previous
"""KroneckerLinear Trainium2 kernel.

Math: out = x @ kron(f1, f2).T + bias, with x [64, 8192], f1 [128,128],
f2 [64,64], bias [8192].  Kronecker identity:
    out[b].reshape(128, 64) = f1 @ X_b @ f2.T,   X_b = x[b].reshape(128, 64)
so the 8192x8192 weight (256 MB) is never materialized; the kernel is
memory-bound on x in / out (~4 MB total).

Sharding: batch-parallel over the 8 NeuronCores, 8 batch rows per core.

Per-core device program (SPMD, identical on all cores):
  loads: matmul operands in bf16 (halves DMA bytes; PE runs 1 cycle/row),
     bias fp32 (riding in the bf16 tensor via a bitcast view; the final
     add runs in fp32 against the fp32 PSUM).  Two parallel DMAs on the
     two HWDGE rings (sync + scalar), each fully contiguous; per-DMA
     completion latency (~1.5us) dominates, so exactly two big DMAs beat
     any finer split.
     A = blk | f1t | xt0 | xt1, B = xt2 | xt3 | biasr.
     xt[h*64+l, p*128+j] = x[lb, j*64+l] for local batch lb = p + 4h;
     blk = blkdiag(f2.T, f2.T) so one K=128 matmul computes TWO batches.
  stage 1 (apply f2): matmul p: lhsT = xt_p, rhs = blk ->
     psum_v[p][j, h*64+k] = (X_{p+4h} @ f2.T)[j, k], one PSUM tile per p
     so each PSUM->SBUF cast only waits on its own matmul.  Casts
     (fp32 PSUM -> bf16 v) alternate scalar (ACTIVATE) / vector (DVE);
     gpsimd can't read PSUM on TRN2.
  stage 2 (apply f1): two matmuls lhsT = f1t, rhs = v half [128, 256].
  bias: fused with the PSUM->SBUF move on DVE, fp32.
  stores: two contiguous fp32 DMAs (sync + scalar rings in parallel) of
     the device-natural layout y[i, p*128+h*64+k]; host unpermutes.
"""

import numpy as np

N_CORES = 8
B = 64
LB = B // N_CORES  # 8 local batches per core

_CACHE = {}


def _build_nc():
    import concourse.bass as bass
    import concourse.mybir as mybir
    import concourse.tile as tile
    from concourse import bacc

    fp32 = mybir.dt.float32
    bf16 = mybir.dt.bfloat16

    nc = bacc.Bacc("TRN2", target_bir_lowering=False, debug=False)
    # contiguous per-ring inputs:
    # A: blk 0:128 | f1t 128:256 | xt0 256:384 | xt1 384:512   (bf16)
    # B: xt2 0:128 | xt3 128:256 | biasr-as-bf16 256:384       (bf16)
    inA_d = nc.dram_tensor("inpA", [128, 512], bf16, kind="ExternalInput")
    inB_d = nc.dram_tensor("inpB", [128, 384], bf16, kind="ExternalInput")
    y0_d = nc.dram_tensor("y0", [128, 256], bf16, kind="ExternalOutput")
    y1_d = nc.dram_tensor("y1", [128, 256], bf16, kind="ExternalOutput")

    with tile.TileContext(nc) as tc:
        with (
            tc.tile_pool(name="sb", bufs=1) as sb,
            tc.tile_pool(name="psv", bufs=1, space="PSUM") as psv,
            tc.tile_pool(name="psy", bufs=1, space="PSUM") as psy,
        ):
            sbA = sb.tile([128, 512], bf16)
            sbB = sb.tile([128, 384], bf16)
            v = sb.tile([128, 512], bf16)
            out0 = sb.tile([128, 256], bf16)
            out1 = sb.tile([128, 256], bf16)
            warm = sb.tile([128, 128], bf16)
            warm_v = sb.tile([128, 128], bf16)

            blk = sbA[:, 0:128]
            f1t = sbA[:, 128:256]
            biasr = sbB[:, 256:384].bitcast(fp32)  # [128, 64] fp32

            def xt_slice(p):
                return (
                    sbA[:, 256 + p * 128 : 384 + p * 128]
                    if p < 2
                    else sbB[:, (p - 2) * 128 : (p - 1) * 128]
                )

            # two parallel input DMAs, one per HWDGE ring
            nc.sync.dma_start(out=sbA[:, :], in_=inA_d[:, :])
            nc.scalar.dma_start(out=sbB[:, :], in_=inB_d[:, :])

            # warm-up: keep PE/DVE clocks ramped through the ~2.3us DMA-wait
            # window so the first real ops run at full rate, sized to end
            # just before the input lands
            nc.vector.memset(warm[:, :], 0.0)
            pw = psv.tile([128, 128], fp32, tag="psum_warm")
            for _ in range(4):
                nc.tensor.matmul(pw[:], warm[:, :], warm[:, :], start=True, stop=True)
            for _ in range(3):
                nc.vector.tensor_copy(warm_v[:, :], warm[:, :])



            b_ap = biasr
            bias_bcast = bass.AP(
                tensor=b_ap.tensor,
                offset=b_ap.offset,
                ap=[b_ap.ap[0], [0, 4], b_ap.ap[1]],
            )

            # stage 1: 4 matmuls, paired into two PSUM tiles so each
            # PSUM->SBUF cast covers a [128, 256] pair in one DVE op
            psum_v = []
            for pair in range(2):
                pv = psv.tile([128, 256], fp32, tag=f"psum_v{pair}")
                psum_v.append(pv)
                for half in range(2):
                    p = pair * 2 + half
                    nc.tensor.matmul(
                        pv[:, half * 128 : (half + 1) * 128],
                        xt_slice(p),
                        blk,
                        start=True,
                        stop=True,
                    )
            # PSUM -> SBUF casts (fp32 -> bf16), all on DVE: using scalar
            # ACTIVATE would pull in an act-table DMA that contends with the
            # scalar-ring input DMA for HBM bandwidth
            for pair in range(2):
                dst = v[:, pair * 256 : (pair + 1) * 256]
                nc.vector.tensor_copy(dst, psum_v[pair][:])

            # stage 2: two N=256 matmuls
            psum_y = []
            for hlf in range(2):
                py = psy.tile([128, 256], fp32, tag=f"psum_y{hlf}")
                psum_y.append(py)
                nc.tensor.matmul(
                    py[:],
                    f1t,
                    v[:, hlf * 256 : (hlf + 1) * 256],
                    start=True,
                    stop=True,
                )

            # bias add fused with PSUM->SBUF move (DVE, fp32)
            for hlf, out_sb in enumerate([out0, out1]):
                o_ap = out_sb[:, :]
                out_g = bass.AP(
                    tensor=o_ap.tensor,
                    offset=o_ap.offset,
                    ap=[o_ap.ap[0], [64, 4], [1, 64]],
                )
                y_ap = psum_y[hlf][:, :]
                y_g = bass.AP(
                    tensor=y_ap.tensor,
                    offset=y_ap.offset,
                    ap=[y_ap.ap[0], [64, 4], [1, 64]],
                )
                nc.vector.tensor_add(out_g, y_g, bias_bcast)

            # two parallel contiguous output DMAs; the later half rides the
            # sync ring, whose completion latency is the more reliable
            nc.scalar.dma_start(out=y0_d[:, :], in_=out0[:, :])
            nc.sync.dma_start(out=y1_d[:, :], in_=out1[:, :])

    nc.compile()
    return nc


def _prep_core_inputs(x, factor1, factor2, bias):
    """Host-side layout prep. Returns list of per-core in_maps."""
    import ml_dtypes

    bf16 = ml_dtypes.bfloat16
    x = np.ascontiguousarray(np.asarray(x, dtype=np.float32))
    f1 = np.asarray(factor1, dtype=np.float32)
    f2 = np.asarray(factor2, dtype=np.float32)
    bias = np.asarray(bias, dtype=np.float32)

    # x -> per-core xt [128, 512]: xt[h*64+l, p*128+j] = x[c*8 + p + 4h, j*64+l]
    xc = x.reshape(N_CORES, LB, 128, 64)  # [c, lb, j, l]
    arr = xc.transpose(0, 3, 1, 2).reshape(N_CORES, 64, 2, 4, 128)
    xt_all = arr.transpose(0, 2, 1, 3, 4).reshape(N_CORES, 128, 512).astype(bf16)

    inA = np.zeros((N_CORES, 128, 512), dtype=bf16)
    f2t = f2.T.astype(bf16)
    inA[:, :64, 0:64] = f2t
    inA[:, 64:, 64:128] = f2t
    inA[:, :, 128:256] = f1.T.astype(bf16)
    inA[:, :, 256:512] = xt_all[:, :, 0:256]
    inB = np.empty((N_CORES, 128, 384), dtype=bf16)
    inB[:, :, 0:256] = xt_all[:, :, 256:512]
    bias_bf = np.ascontiguousarray(bias.reshape(128, 64)).view(bf16)  # [128,128]
    inB[:, :, 256:384] = bias_bf[None]

    return [
        {"inpA": np.ascontiguousarray(inA[c]), "inpB": np.ascontiguousarray(inB[c])}
        for c in range(N_CORES)
    ]


def kernel(x, factor1, factor2, bias):
    from concourse.bass_utils import run_bass_kernel_spmd

    if "nc" not in _CACHE:
        _CACHE["nc"] = _build_nc()
    nc = _CACHE["nc"]

    in_maps = _prep_core_inputs(x, factor1, factor2, bias)
    res = run_bass_kernel_spmd(nc, in_maps, core_ids=list(range(N_CORES)))
    kernel.last_results = res

    # device layout: y[i, p*128 + h*64 + k] = out[c*8 + p + 4h, i*64 + k]
    # row order after reshape is r = 2p + h; batch lb = p + 4h -> inv perm
    inv = np.array([0, 2, 4, 6, 1, 3, 5, 7])
    outs = []
    for c in range(N_CORES):
        yc = np.concatenate(
            [res.results[c]["y0"], res.results[c]["y1"]], axis=1
        ).astype(np.float32)  # [128, 512], device writes bf16
        yc = yc.reshape(128, 4, 2, 64).transpose(1, 2, 0, 3).reshape(8, 8192)
        outs.append(yc[inv])
    return np.concatenate(outs, axis=0)
